# revision 3
# baseline (speedup 1.0000x reference)
"""Trainium2 Bass kernel for a 5-member ensemble dynamics MLP.

Model: per ensemble e, x[e] @ w0[e]+b0 -> silu -> (200x200 silu) x3 ->
w4[e]+b4 -> split (mean, logvar) -> double softplus clamp of logvar.

Sharding: pure data parallel over the batch dim (65536 -> 8 x 8192);
the ~1.4 MB of ensemble weights is replicated to every core.

v3 layout notes:
- All matmul operands are bfloat16 (weights, x, hidden activations).
  fp32r streams SBUF at ~2 cycles/row on HW; bf16 is full rate. PSUM
  accumulates fp32; rel tolerance (2e-2) dwarfs bf16 noise (~5e-3).
- Output layer is packed to a single stationary block: W4' = [mean(31)
  | zero pad(1) | logvar(31)] so one (K-block) pair of matmuls yields a
  [63, NT] PSUM tile; mean reads partitions 0:31, logvar 32:63 (both
  32-aligned). Saves 2048 PE rows/tile vs separate mean/logvar matmuls.
- DMA discipline: descriptor generation occupies the issuing engine's
  queue (~40ns/partition row), so big/hot transfers are spread across
  queues: inputs + mean output on sync, logvar output on gpsimd. Mean
  accumulates into one persistent [31, BS] SBUF buffer per ensemble ->
  a single output DMA per ensemble instead of one per tile. Weight
  loads are emitted per-ensemble, ordered w0 -> x -> rest, so the first
  matmul can start within a few microseconds.
- logvar clamp (phase 2) uses the exact identity
    out = min + ln(C2 + t) - ln(1 + t),  t = e^{max - lv},
    C2 = 1 + e^{max - min}
  (one Exp + two Ln, same act table set, + one DVE op). It is emitted
  per-ensemble right after that ensemble's tiles: the tile scheduler
  prioritizes by program order, so each ensemble boundary costs ~3 act
  table switches instead of thrashing Silu<->Exp/Ln per instruction.
- Raw logvar rows are staged packed 4-tiles-up (31 rows per 32-stride
  group) so phase-2 activations run ~124/128 full partitions.
"""

import sys

if "/opt/trn_rl_repo" not in sys.path:
    sys.path.insert(0, "/opt/trn_rl_repo")

import numpy as np

E = 5
B = 65536
IN_DIM = 38
H = 200
OUT = 31  # mean / logvar feature count
NCORES = 8
BS = B // NCORES  # samples per core
NT = 1024  # batch-tile columns
NTILES = BS // NT
K0 = 128
K1 = H - K0  # 72
M4 = 2 * OUT + 1  # packed L4 output block: mean | pad | logvar
PACK = 4  # logvar tiles packed per partition group in phase 2
RSTRIDE = 32  # partition stride per packed tile
P2P = PACK * RSTRIDE  # 128 partitions, top row of each 32-group unused
P2N = 1024  # phase-2 Ln/DVE free-dim chunk

_CACHE = {}


def _build():
    import concourse.bass as bass  # noqa: F401
    import concourse.tile as tile
    from concourse import bacc, mybir
    from contextlib import ExitStack

    fp32 = mybir.dt.float32
    bf16 = mybir.dt.bfloat16
    AF = mybir.ActivationFunctionType
    ALU = mybir.AluOpType

    nc = bacc.Bacc("TRN2", target_bir_lowering=False, debug=False)

    xT = nc.dram_tensor("xT", [E, IN_DIM, BS], bf16, kind="ExternalInput").ap()
    w_d = [
        nc.dram_tensor("w0", [E, IN_DIM, H], bf16, kind="ExternalInput").ap(),
        nc.dram_tensor("w1", [E, H, H], bf16, kind="ExternalInput").ap(),
        nc.dram_tensor("w2", [E, H, H], bf16, kind="ExternalInput").ap(),
        nc.dram_tensor("w3", [E, H, H], bf16, kind="ExternalInput").ap(),
        nc.dram_tensor("w4p", [E, H, M4], bf16, kind="ExternalInput").ap(),
    ]
    b_d = [
        nc.dram_tensor(f"b{l}", [E, H, 1], fp32, kind="ExternalInput").ap()
        for l in range(4)
    ]
    b4m_d = nc.dram_tensor("b4m", [E, OUT, 1], fp32, kind="ExternalInput").ap()
    # phase-2 per-partition constants, pre-tiled to the packed 128 rows
    c1_d = nc.dram_tensor("c1", [E, P2P, 1], fp32, kind="ExternalInput").ap()
    c2_d = nc.dram_tensor("c2", [P2P, 1], fp32, kind="ExternalInput").ap()
    minlv_d = nc.dram_tensor("minlv", [P2P, 1], fp32, kind="ExternalInput").ap()
    om_d = nc.dram_tensor("out_mean", [E, OUT, BS], fp32, kind="ExternalOutput").ap()
    ol_d = nc.dram_tensor("out_logvar", [E, OUT, BS], fp32, kind="ExternalOutput").ap()

    with tile.TileContext(nc) as tc, ExitStack() as ctx:
        wpool = ctx.enter_context(tc.tile_pool(name="wts", bufs=1))
        stpool = ctx.enter_context(tc.tile_pool(name="stage", bufs=1))
        xpool = ctx.enter_context(tc.tile_pool(name="x", bufs=2))
        hpool = ctx.enter_context(tc.tile_pool(name="h", bufs=4))
        pspool = ctx.enter_context(tc.tile_pool(name="ps", bufs=2, space="PSUM"))
        mpool = ctx.enter_context(tc.tile_pool(name="mean", bufs=1))
        tpool = ctx.enter_context(tc.tile_pool(name="p2t", bufs=2))
        p2pool = ctx.enter_context(tc.tile_pool(name="p2", bufs=2))

        W = {}

        def _const(tag, shape, src, dt=fp32):
            t = wpool.tile(shape, dt, tag=tag)
            nc.sync.dma_start(t[:], src)
            W[tag] = t
            return t

        # global phase-2 constants
        c2 = _const("c2", [P2P, 1], c2_d[:])
        minlv = _const("minlv", [P2P, 1], minlv_d[:])

        # raw-logvar staging buffers, one per ensemble, packed 4-tiles-up
        stage = []
        for e in range(E):
            st = stpool.tile(
                [P2P, NTILES // PACK * NT], fp32, tag=f"stage_{e}", name=f"stage_{e}"
            )
            nc.vector.memset(st[:], 0.0)
            stage.append(st)

        def mm(ps, lhsT, rhs, start, stop):
            for c0 in range(0, rhs.shape[-1], 512):
                nc.tensor.matmul(
                    ps[:, c0 : c0 + 512],
                    lhsT,
                    rhs[:, c0 : c0 + 512],
                    start=start,
                    stop=stop,
                )

        ncol = NTILES // PACK * NT  # staged cols per ensemble

        for e in range(E):
            # ---- per-ensemble loads, ordered so tile 0 can start ASAP ----
            _const(f"w0_{e}", [IN_DIM, H], w_d[0][e], bf16)
            _const(f"b0a_{e}", [K0, 1], b_d[0][e, 0:K0, :])
            _const(f"b0b_{e}", [K1, 1], b_d[0][e, K0:H, :])
            xe = xpool.tile([IN_DIM, BS], bf16, tag="x")
            nc.sync.dma_start(xe[:], xT[e])
            for l in (1, 2, 3):
                _const(f"w{l}a_{e}", [K0, H], w_d[l][e, 0:K0, :], bf16)
                _const(f"w{l}b_{e}", [K1, H], w_d[l][e, K0:H, :], bf16)
                _const(f"b{l}a_{e}", [K0, 1], b_d[l][e, 0:K0, :])
                _const(f"b{l}b_{e}", [K1, 1], b_d[l][e, K0:H, :])
            _const(f"w4a_{e}", [K0, M4], w_d[4][e, 0:K0, :], bf16)
            _const(f"w4b_{e}", [K1, M4], w_d[4][e, K0:H, :], bf16)
            _const(f"b4m_{e}", [OUT, 1], b4m_d[e])
            _const(f"c1_{e}", [P2P, 1], c1_d[e])

            meanbuf = mpool.tile([OUT, BS], fp32, tag="meanbuf")

            # ---- MLP tiles (Silu table) ----
            for t in range(NTILES):
                cs = slice(t * NT, (t + 1) * NT)

                # layer 0: K=38, M=200 (128+72)
                pa = pspool.tile([K0, NT], fp32, tag="psa")
                pb = pspool.tile([K1, NT], fp32, tag="psb")
                mm(pa[:], W[f"w0_{e}"][:, 0:K0], xe[:, cs], True, True)
                mm(pb[:], W[f"w0_{e}"][:, K0:H], xe[:, cs], True, True)
                ha = hpool.tile([K0, NT], bf16, tag="ha")
                hb = hpool.tile([K1, NT], bf16, tag="hb")
                nc.scalar.activation(ha[:], pa[:], AF.Silu, bias=W[f"b0a_{e}"][:])
                nc.scalar.activation(hb[:], pb[:], AF.Silu, bias=W[f"b0b_{e}"][:])

                # layers 1..3: K=200 (128+72), M=200 (128+72)
                for l in (1, 2, 3):
                    pa = pspool.tile([K0, NT], fp32, tag="psa")
                    pb = pspool.tile([K1, NT], fp32, tag="psb")
                    mm(pa[:], W[f"w{l}a_{e}"][:, 0:K0], ha[:], True, False)
                    mm(pa[:], W[f"w{l}b_{e}"][:, 0:K0], hb[:], False, True)
                    mm(pb[:], W[f"w{l}a_{e}"][:, K0:H], ha[:], True, False)
                    mm(pb[:], W[f"w{l}b_{e}"][:, K0:H], hb[:], False, True)
                    ha = hpool.tile([K0, NT], bf16, tag="ha")
                    hb = hpool.tile([K1, NT], bf16, tag="hb")
                    nc.scalar.activation(ha[:], pa[:], AF.Silu, bias=W[f"b{l}a_{e}"][:])
                    nc.scalar.activation(hb[:], pb[:], AF.Silu, bias=W[f"b{l}b_{e}"][:])

                # layer 4: K=200, single packed M=63 block (mean | pad | logvar)
                pm = pspool.tile([M4, NT], fp32, tag="psa")
                mm(pm[:], W[f"w4a_{e}"][:], ha[:], True, False)
                mm(pm[:], W[f"w4b_{e}"][:], hb[:], False, True)

                nc.vector.tensor_scalar_add(
                    meanbuf[:, cs], pm[0:OUT, :], W[f"b4m_{e}"][:]
                )

                # stash raw logvar rows: tile t -> rows 32*(t%4), cols 1024*(t//4)
                r = (t % PACK) * RSTRIDE
                c = (t // PACK) * NT
                nc.vector.tensor_copy(
                    stage[e][r : r + OUT, c : c + NT], pm[OUT + 1 : M4, :]
                )

            nc.sync.dma_start(om_d[e], meanbuf[:])

            # ---- logvar clamp (Exp/Ln table), inline per ensemble ----
            # lv = z + b4lv (bias folded into c1 = max - b4lv):
            #   t   = Exp(-z + c1) = e^{max - lv}
            #   out = min + Ln(t + C2) - Ln(t + 1),  C2 = 1 + e^{max - min}
            te = tpool.tile([P2P, ncol], fp32, tag="p2t")
            nc.scalar.activation(
                te[:], stage[e][:], AF.Exp, bias=W[f"c1_{e}"][:], scale=-1.0
            )
            for g in range(ncol // P2N):
                gs = slice(g * P2N, (g + 1) * P2N)
                a = p2pool.tile([P2P, P2N], fp32, tag="p2a")
                nc.scalar.activation(a[:], te[:, gs], AF.Ln, bias=c2[:])
                b = p2pool.tile([P2P, P2N], fp32, tag="p2b")
                nc.scalar.activation(b[:], te[:, gs], AF.Ln, bias=1.0)
                lvo = p2pool.tile([P2P, P2N], fp32, tag="p2c")
                # (a + min) - b
                nc.vector.scalar_tensor_tensor(
                    lvo[:], a[:], minlv[:], b[:], ALU.add, ALU.subtract
                )
                # unpack via the (otherwise idle) gpsimd DMA queue
                for j in range(P2N // NT):
                    col = g * P2N + j * NT
                    tcol = col // NT  # global col-block index = t // PACK
                    for r in range(PACK):
                        t = tcol * PACK + r
                        nc.gpsimd.dma_start(
                            ol_d[e, :, t * NT : (t + 1) * NT],
                            lvo[r * RSTRIDE : r * RSTRIDE + OUT, j * NT : (j + 1) * NT],
                        )

    nc.compile()
    return nc


def _prep_host(x, w0, b0, w1, b1, w2, b2, w3, b3, w4, b4, max_logvar, min_logvar):
    import ml_dtypes

    f = np.float32
    bf = ml_dtypes.bfloat16
    b4f = np.asarray(b4, f).reshape(E, 2 * OUT)
    w4f = np.asarray(w4, f)
    w4p = np.zeros((E, H, M4), f)
    w4p[:, :, 0:OUT] = w4f[:, :, 0:OUT]
    w4p[:, :, OUT + 1 : M4] = w4f[:, :, OUT : 2 * OUT]
    common = {
        "w0": np.ascontiguousarray(np.asarray(w0, f).astype(bf)),
        "w1": np.ascontiguousarray(np.asarray(w1, f).astype(bf)),
        "w2": np.ascontiguousarray(np.asarray(w2, f).astype(bf)),
        "w3": np.ascontiguousarray(np.asarray(w3, f).astype(bf)),
        "w4p": np.ascontiguousarray(w4p.astype(bf)),
    }
    for l, b in enumerate((b0, b1, b2, b3)):
        common[f"b{l}"] = np.ascontiguousarray(np.asarray(b, f).reshape(E, H, 1))
    common["b4m"] = np.ascontiguousarray(b4f[:, :OUT].reshape(E, OUT, 1))
    mx = np.asarray(max_logvar, f).reshape(OUT)
    mn = np.asarray(min_logvar, f).reshape(OUT)
    c1 = mx[None, :] - b4f[:, OUT:]  # [E, 31]
    c2 = 1.0 + np.exp(mx - mn)  # [31]

    def _pack31(v, pad=0.0):  # [..., 31] -> [..., PACK*32, 1] with pad rows
        out = np.full(v.shape[:-1] + (PACK, RSTRIDE), pad, f)
        out[..., :, :OUT] = v[..., None, :]
        return out.reshape(v.shape[:-1] + (P2P, 1))

    common["c1"] = np.ascontiguousarray(_pack31(c1))
    common["c2"] = np.ascontiguousarray(_pack31(c2, pad=1.0))
    common["minlv"] = np.ascontiguousarray(_pack31(mn))

    xf = np.asarray(x, f)
    in_maps = []
    for c in range(NCORES):
        xc = np.ascontiguousarray(
            xf[:, c * BS : (c + 1) * BS, :].transpose(0, 2, 1).astype(bf)
        )
        in_maps.append({"xT": xc, **common})
    return in_maps


def _run(inputs, trace=False):
    from concourse.bass_utils import run_bass_kernel_spmd

    if "nc" not in _CACHE:
        _CACHE["nc"] = _build()
    nc = _CACHE["nc"]
    in_maps = _prep_host(**inputs)
    res = run_bass_kernel_spmd(nc, in_maps, core_ids=list(range(NCORES)), trace=trace)
    mean = np.concatenate(
        [res.results[c]["out_mean"].transpose(0, 2, 1) for c in range(NCORES)], axis=1
    )
    logvar = np.concatenate(
        [res.results[c]["out_logvar"].transpose(0, 2, 1) for c in range(NCORES)],
        axis=1,
    )
    return (mean, logvar), res


def kernel(**inputs):
    out, _ = _run(inputs, trace=False)
    return out
